# revision 2
# baseline (speedup 1.0000x reference)
"""Trainium2 Bass kernel for nn_LAT_16260746182953.

Strategy (8 NeuronCores):
- The dominant cost is the 4 per-level Linears: [672, K_l] @ [K_l, 512] with
  K_l = c*49 up to 100352 (129.5 GFLOP, ~0.9 GB of operands).
- Shard the CONTRACTION dim K 8-ways: each core streams 1/8 of x (transposed,
  bf16) and 1/8 of W (transposed, bf16) and accumulates partial [672, 512]
  sums in PSUM (fp32), then a ReduceScatter(add) sums partials across cores
  and hands core i exactly the rows of batch b=i (672 = 8 batches x 84 rows).
  Per-core traffic: ~56 MB vs ~450 MB for batch-data-parallel.
- All DMAs are issued on gpsimd (SWDGE): HWDGE (nc.sync) DMAs coexisting with
  collectives in one NEFF crash the device (measured).
- Host-side: input layout prep (transpose to [K, 672] p-major chunk blocks,
  bf16 cast) and the tiny epilogue (LayerNorm + two 4-token attentions +
  refine MLP, <0.5% of FLOPs) in exact fp32 numpy.
"""
import numpy as np
import ml_dtypes

import concourse.bacc as bacc
import concourse.mybir as mybir
import concourse.tile as tile
from concourse.bass_utils import run_bass_kernel_spmd

NCORES = 8
B, V, J = 8, 4, 21
E = 512
NH = 8
M = B * V * J          # 672 rows of the big matmul
SLAB = M // NCORES     # 84 rows per core after reduce-scatter
CH = 16                # k-tiles (of 128 rows) per DMA chunk
CHANNELS = [256, 512, 1024, 2048]
KFULL = [c * 49 for c in CHANNELS]
KS = [k // NCORES for k in KFULL]           # per-core contraction shard
NT = [(k + 127) // 128 for k in KS]         # zero-padded k-tiles
LEVELS = [3, 2, 1, 0]                       # biggest level first
MT = [(0, 128), (128, 128), (256, 128), (384, 128), (512, 128), (640, 32)]
BF16 = ml_dtypes.bfloat16


def _chunks_of(nt):
    out, o = [], 0
    while o < nt:
        c = min(CH, nt - o)
        out.append((o, c))
        o += c
    return out


_STATE = {}


def _build():
    nc = bacc.Bacc("TRN2", target_bir_lowering=False, debug=False, num_devices=NCORES)
    x_in = {l: nc.dram_tensor(f"x{l}", [NT[l] * 128 * M], mybir.dt.bfloat16, kind="ExternalInput") for l in range(4)}
    w_in = {l: nc.dram_tensor(f"w{l}", [NT[l] * 128 * E], mybir.dt.bfloat16, kind="ExternalInput") for l in range(4)}
    zred = nc.dram_tensor("zred", [4 * SLAB, E], mybir.dt.float32, kind="ExternalOutput")

    with tile.TileContext(nc) as tc:
        with (
            tc.tile_pool(name="xp", bufs=3) as xp,
            tc.tile_pool(name="wp", bufs=3) as wp,
            tc.tile_pool(name="zp", bufs=2) as zp,
            tc.tile_pool(name="acc", bufs=1, space="PSUM") as accp,
            tc.tile_pool(name="dram", bufs=1, space="DRAM") as dram,
        ):
            for l in LEVELS:
                cks = _chunks_of(NT[l])
                psums = [
                    accp.tile([mw, E], mybir.dt.float32, tag=f"acc{mi}", name=f"acc{l}_{mi}")
                    for mi, (mo, mw) in enumerate(MT)
                ]
                for ci, (t0, ct) in enumerate(cks):
                    xt = xp.tile([128, ct * M], mybir.dt.bfloat16, tag="x", name=f"x_{l}_{ci}")
                    nc.gpsimd.dma_start(
                        out=xt[:],
                        in_=x_in[l].ap()[t0 * 128 * M:(t0 + ct) * 128 * M].rearrange("(p f) -> p f", p=128),
                    )
                    wt = wp.tile([128, ct * E], mybir.dt.bfloat16, tag="w", name=f"w_{l}_{ci}")
                    nc.gpsimd.dma_start(
                        out=wt[:],
                        in_=w_in[l].ap()[t0 * 128 * E:(t0 + ct) * 128 * E].rearrange("(p f) -> p f", p=128),
                    )
                    first, last = ci == 0, ci == len(cks) - 1
                    for mi, (mo, mw) in enumerate(MT):
                        for kt in range(ct):
                            nc.tensor.matmul(
                                psums[mi][:],
                                xt[:, kt * M + mo: kt * M + mo + mw],
                                wt[:, kt * E:(kt + 1) * E],
                                start=(first and kt == 0),
                                stop=(last and kt == ct - 1),
                            )
                cc_in = dram.tile([M, E], mybir.dt.float32, tag="cc_in", name=f"cc_in{l}", bufs=2)
                for mi, (mo, mw) in enumerate(MT):
                    zt = zp.tile([mw, E], mybir.dt.float32, tag="z", name=f"z_{l}_{mi}")
                    nc.vector.tensor_copy(zt[:], psums[mi][:])
                    nc.gpsimd.dma_start(out=cc_in[mo:mo + mw, :], in_=zt[:])
                cc_out = dram.tile([SLAB, E], mybir.dt.float32, tag="cc_out", name=f"cc_out{l}", bufs=2)
                nc.gpsimd.collective_compute(
                    "ReduceScatter",
                    mybir.AluOpType.add,
                    replica_groups=[list(range(NCORES))],
                    ins=[cc_in.opt()],
                    outs=[cc_out.opt()],
                )
                nc.gpsimd.dma_start(out=zred.ap()[l * SLAB:(l + 1) * SLAB, :], in_=cc_out[:])

    nc.compile()
    return nc


def _prep(a_pad, F):
    """[nt*128, F] f32/bf16 -> flat chunk-major, p-major bf16 layout."""
    nt = a_pad.shape[0] // 128
    out = []
    for t0, ct in _chunks_of(nt):
        blk = a_pad[t0 * 128:(t0 + ct) * 128].reshape(ct, 128, F).swapaxes(0, 1)
        out.append(np.ascontiguousarray(blk).reshape(-1))
    return np.concatenate(out)


def _prep_inputs(jaf, level_params):
    in_maps = [{} for _ in range(NCORES)]
    for l in range(4):
        K = KFULL[l]
        x2 = np.asarray(jaf[l], np.float32).reshape(M, K)
        Wl = np.asarray(level_params[l][0], np.float32)  # [E, K]
        for i in range(NCORES):
            sl = slice(i * KS[l], (i + 1) * KS[l])
            xpad = np.zeros((NT[l] * 128, M), BF16)
            xpad[:KS[l]] = x2[:, sl].T
            in_maps[i][f"x{l}"] = _prep(xpad, M)
            wpad = np.zeros((NT[l] * 128, E), BF16)
            wpad[:KS[l]] = Wl[:, sl].T
            in_maps[i][f"w{l}"] = _prep(wpad, E)
    return in_maps


# ---------------- host epilogue (exact fp32, mirrors the reference) ----------------

def _layernorm(x, g, b):
    mu = x.mean(-1, keepdims=True)
    var = ((x - mu) ** 2).mean(-1, keepdims=True)
    return (x - mu) / np.sqrt(var + 1e-5) * g + b


def _mha(x, Wi, bi, Wo, bo):
    S, N, _ = x.shape
    hd = E // NH
    qkv = x @ Wi.T + bi
    q, k, v = np.split(qkv, 3, axis=-1)

    def heads(t):
        return t.reshape(S, N, NH, hd).transpose(1, 2, 0, 3)

    q, k, v = heads(q), heads(k), heads(v)
    scores = np.einsum('nhqd,nhkd->nhqk', q, k) / np.sqrt(np.float32(hd))
    scores = scores - scores.max(-1, keepdims=True)
    ex = np.exp(scores)
    attn = ex / ex.sum(-1, keepdims=True)
    o = np.einsum('nhqk,nhkd->nhqd', attn, v)
    o = o.transpose(2, 0, 1, 3).reshape(S, N, E)
    return o @ Wo.T + bo


def kernel(jaf_64, jaf_32, jaf_16, jaf_8, joints_3d, level_params, view_attn, level_attn, refine_params):
    jaf = [jaf_64, jaf_32, jaf_16, jaf_8]
    level_params = [tuple(np.asarray(t, np.float32) for t in tp) for tp in level_params]
    view_attn = tuple(np.asarray(t, np.float32) for t in view_attn)
    level_attn = tuple(np.asarray(t, np.float32) for t in level_attn)
    W1, b1, W2, b2 = (np.asarray(t, np.float32) for t in refine_params)
    joints = np.asarray(joints_3d, np.float32)

    if "nc" not in _STATE:
        _STATE["nc"] = _build()
    nc = _STATE["nc"]

    in_maps = _prep_inputs(jaf, level_params)
    _STATE["in_maps"] = in_maps
    res = run_bass_kernel_spmd(nc, in_maps, list(range(NCORES)))
    _STATE["res"] = res

    out = np.empty((B, J, 3), np.float32)
    for b in range(NCORES):
        zred = res.results[b]["zred"]  # [4*84, 512] summed partials, batch b
        level_out = []
        for l in range(4):
            _, bias, g, beta = level_params[l]
            z = zred[l * SLAB:(l + 1) * SLAB] + bias
            h = _layernorm(np.maximum(z, 0.0), g, beta)
            o = _mha(h.reshape(V, J, E), *view_attn)
            level_out.append(o.mean(0))
        L = np.stack(level_out, 0)                      # [4, J, E]
        fused = _mha(L, *level_attn).mean(0)            # [J, E]
        delta = np.maximum(fused @ W1.T + b1, 0.0) @ W2.T + b2
        out[b] = joints[b] + delta
    return out


def hw_exec_time_ns():
    """Best-effort HW time: NTFF trace if available, else min warm wall time
    of the SPMD execution (includes host<->device transfer, so upper bound)."""
    import time
    try:
        res = run_bass_kernel_spmd(_STATE["nc"], _STATE["in_maps"], list(range(NCORES)), trace=True)
        if res.exec_time_ns:
            path = res.instructions_and_trace[1] if res.instructions_and_trace else None
            return res.exec_time_ns, path
    except Exception:
        pass
    best = None
    for _ in range(3):
        t0 = time.perf_counter()
        run_bass_kernel_spmd(_STATE["nc"], _STATE["in_maps"], list(range(NCORES)))
        dt = time.perf_counter() - t0
        best = dt if best is None or dt < best else best
    return int(best * 1e9), "wall-clock of warm SPMD run (upper bound, incl transfers)"


# revision 10
# speedup vs baseline: 1.0168x; 1.0168x over previous
"""Trainium2 Bass kernel for nn_LAT_16260746182953.

Strategy (8 NeuronCores):
- The dominant cost is the 4 per-level Linears: [672, K_l] @ [K_l, 512] with
  K_l = c*49 up to 100352 (129.5 GFLOP, ~0.9 GB of operands).
- Shard the CONTRACTION dim K 8-ways: each core streams 1/8 of x (transposed,
  bf16) and 1/8 of W (transposed, bf16) and accumulates partial [672, 512]
  sums in PSUM (fp32), then a ReduceScatter(add) sums partials across cores
  and hands core i exactly the rows of batch b=i (672 = 8 batches x 84 rows).
  Per-core traffic: ~56 MB vs ~450 MB for batch-data-parallel.
- All DMAs are issued on gpsimd (SWDGE): HWDGE (nc.sync) DMAs coexisting with
  collectives in one NEFF crash the device (measured).
- Host-side: input layout prep (transpose to [K, 672] p-major chunk blocks,
  bf16 cast) and the tiny epilogue (LayerNorm + two 4-token attentions +
  refine MLP, <0.5% of FLOPs) in exact fp32 numpy.
"""
import numpy as np
import ml_dtypes

import concourse.bacc as bacc
import concourse.mybir as mybir
import concourse.tile as tile
from concourse.bass_utils import run_bass_kernel_spmd
from concourse import bass2jax

NCORES = 8
B, V, J = 8, 4, 21
E = 512
NH = 8
M = B * V * J          # 672 rows of the big matmul
SLAB = M // NCORES     # 84 rows per core after reduce-scatter
CH = 16                # k-tiles (of 128 rows) per DMA chunk
CHANNELS = [256, 512, 1024, 2048]
KFULL = [c * 49 for c in CHANNELS]
KS = [k // NCORES for k in KFULL]           # per-core contraction shard
NT = [(k + 127) // 128 for k in KS]         # zero-padded k-tiles
LEVELS = [3, 2, 1, 0]                       # biggest level first
MT = [(0, 128), (128, 128), (256, 128), (384, 128), (512, 128), (640, 32)]
BF16 = ml_dtypes.bfloat16


def _chunks_of(nt):
    out, o = [], 0
    while o < nt:
        c = min(CH, nt - o)
        out.append((o, c))
        o += c
    return out


_STATE = {}


def _build():
    nc = bacc.Bacc("TRN2", target_bir_lowering=False, debug=False, num_devices=NCORES)
    x_in = {l: nc.dram_tensor(f"x{l}", [NT[l] * 128 * M], mybir.dt.bfloat16, kind="ExternalInput") for l in range(4)}
    w_in = {l: nc.dram_tensor(f"w{l}", [NT[l] * 128 * E], mybir.dt.bfloat16, kind="ExternalInput") for l in range(4)}
    zred = nc.dram_tensor("zred", [4 * SLAB, E], mybir.dt.float32, kind="ExternalOutput")

    with tile.TileContext(nc) as tc:
        with (
            tc.tile_pool(name="xp", bufs=3) as xp,
            tc.tile_pool(name="wp", bufs=3) as wp,
            tc.tile_pool(name="zp", bufs=2) as zp,
            tc.tile_pool(name="acc", bufs=1, space="PSUM") as accp,
            tc.tile_pool(name="dram", bufs=1, space="DRAM") as dram,
        ):
            for l in LEVELS:
                cks = _chunks_of(NT[l])
                psums = [
                    accp.tile([mw, E], mybir.dt.float32, tag=f"acc{mi}", name=f"acc{l}_{mi}")
                    for mi, (mo, mw) in enumerate(MT)
                ]
                for ci, (t0, ct) in enumerate(cks):
                    xt = xp.tile([128, ct * M], mybir.dt.bfloat16, tag="x", name=f"x_{l}_{ci}")
                    nc.gpsimd.dma_start(
                        out=xt[:],
                        in_=x_in[l].ap()[t0 * 128 * M:(t0 + ct) * 128 * M].rearrange("(p f) -> p f", p=128),
                    )
                    wt = wp.tile([128, ct * E], mybir.dt.bfloat16, tag="w", name=f"w_{l}_{ci}")
                    nc.gpsimd.dma_start(
                        out=wt[:],
                        in_=w_in[l].ap()[t0 * 128 * E:(t0 + ct) * 128 * E].rearrange("(p f) -> p f", p=128),
                    )
                    first, last = ci == 0, ci == len(cks) - 1
                    for mi, (mo, mw) in enumerate(MT):
                        for kt in range(ct):
                            nc.tensor.matmul(
                                psums[mi][:],
                                xt[:, kt * M + mo: kt * M + mo + mw],
                                wt[:, kt * E:(kt + 1) * E],
                                start=(first and kt == 0),
                                stop=(last and kt == ct - 1),
                            )
                cc_in = dram.tile([M, E], mybir.dt.float32, tag="cc_in", name=f"cc_in{l}", bufs=2)
                for mi, (mo, mw) in enumerate(MT):
                    zt = zp.tile([mw, E], mybir.dt.float32, tag="z", name=f"z_{l}_{mi}")
                    nc.vector.tensor_copy(zt[:], psums[mi][:])
                    nc.gpsimd.dma_start(out=cc_in[mo:mo + mw, :], in_=zt[:])
                cc_out = dram.tile([SLAB, E], mybir.dt.float32, tag="cc_out", name=f"cc_out{l}", bufs=2)
                nc.gpsimd.collective_compute(
                    "ReduceScatter",
                    mybir.AluOpType.add,
                    replica_groups=[list(range(NCORES))],
                    ins=[cc_in.opt()],
                    outs=[cc_out.opt()],
                )
                nc.gpsimd.dma_start(out=zred.ap()[l * SLAB:(l + 1) * SLAB, :], in_=cc_out[:])

    nc.compile()
    return nc


def _get_runner(nc):
    """Cached jitted shard_map runner (mirrors bass2jax.run_bass_via_pjrt) so
    repeated executions reuse the XLA executable and inputs can stay on device."""
    if "runner" in _STATE:
        return _STATE["runner"]
    import jax
    from jax.experimental.shard_map import shard_map
    from jax.sharding import Mesh, PartitionSpec

    bass2jax.install_neuronx_cc_hook()
    in_names, out_names, out_avals, zero_outs = [], [], [], []
    for alloc in nc.m.functions[0].allocations:
        if not isinstance(alloc, mybir.MemoryLocationSet):
            continue
        name = alloc.memorylocations[0].name
        if alloc.kind == "ExternalInput":
            in_names.append(name)
        elif alloc.kind == "ExternalOutput":
            shape = tuple(alloc.tensor_shape)
            dtype = mybir.dt.np(alloc.dtype)
            out_names.append(name)
            out_avals.append(jax.core.ShapedArray(shape, dtype))
            zero_outs.append(np.zeros(shape, dtype))
    n_params, n_outs = len(in_names), len(out_avals)
    all_names = in_names + out_names

    def _body(*args):
        outs = bass2jax._bass_exec_p.bind(
            *args,
            out_avals=tuple(out_avals),
            in_names=tuple(all_names),
            out_names=tuple(out_names),
            lowering_input_output_aliases=(),
            sim_require_finite=True,
            sim_require_nnan=True,
            nc=nc,
        )
        return tuple(outs)

    devices = jax.devices()[:NCORES]
    mesh = Mesh(np.asarray(devices), ("core",))
    spec = PartitionSpec("core")
    sharded = jax.jit(
        shard_map(_body, mesh=mesh, in_specs=(spec,) * (n_params + n_outs),
                  out_specs=(spec,) * n_outs, check_rep=False),
        donate_argnums=tuple(range(n_params, n_params + n_outs)),
        keep_unused=True,
    )
    _STATE["runner"] = (sharded, in_names, out_names, out_avals, zero_outs, mesh, spec)
    return _STATE["runner"]


def _run_device(nc, in_maps):
    import jax
    sharded, in_names, out_names, out_avals, zero_outs, mesh, spec = _get_runner(nc)
    concat_in = [np.concatenate([in_maps[c][n] for c in range(NCORES)], axis=0) for n in in_names]
    concat_zeros = [np.zeros((NCORES * z.shape[0], *z.shape[1:]), z.dtype) for z in zero_outs]
    out_arrs = sharded(*concat_in, *concat_zeros)
    return [
        {n: np.asarray(out_arrs[i]).reshape(NCORES, *out_avals[i].shape)[c]
         for i, n in enumerate(out_names)}
        for c in range(NCORES)
    ]


def _prep(a_pad, F):
    """[nt*128, F] f32/bf16 -> flat chunk-major, p-major bf16 layout."""
    nt = a_pad.shape[0] // 128
    out = []
    for t0, ct in _chunks_of(nt):
        blk = a_pad[t0 * 128:(t0 + ct) * 128].reshape(ct, 128, F).swapaxes(0, 1)
        out.append(np.ascontiguousarray(blk).reshape(-1))
    return np.concatenate(out)


def _prep_inputs(jaf, level_params):
    in_maps = [{} for _ in range(NCORES)]
    for l in range(4):
        K = KFULL[l]
        x2 = np.asarray(jaf[l], np.float32).reshape(M, K)
        Wl = np.asarray(level_params[l][0], np.float32)  # [E, K]
        for i in range(NCORES):
            sl = slice(i * KS[l], (i + 1) * KS[l])
            xpad = np.zeros((NT[l] * 128, M), BF16)
            xpad[:KS[l]] = x2[:, sl].T
            in_maps[i][f"x{l}"] = _prep(xpad, M)
            wpad = np.zeros((NT[l] * 128, E), BF16)
            wpad[:KS[l]] = Wl[:, sl].T
            in_maps[i][f"w{l}"] = _prep(wpad, E)
    return in_maps


# ---------------- host epilogue (exact fp32, mirrors the reference) ----------------

def _layernorm(x, g, b):
    mu = x.mean(-1, keepdims=True)
    var = ((x - mu) ** 2).mean(-1, keepdims=True)
    return (x - mu) / np.sqrt(var + 1e-5) * g + b


def _mha(x, Wi, bi, Wo, bo):
    S, N, _ = x.shape
    hd = E // NH
    qkv = x @ Wi.T + bi
    q, k, v = np.split(qkv, 3, axis=-1)

    def heads(t):
        return t.reshape(S, N, NH, hd).transpose(1, 2, 0, 3)

    q, k, v = heads(q), heads(k), heads(v)
    scores = np.einsum('nhqd,nhkd->nhqk', q, k) / np.sqrt(np.float32(hd))
    scores = scores - scores.max(-1, keepdims=True)
    ex = np.exp(scores)
    attn = ex / ex.sum(-1, keepdims=True)
    o = np.einsum('nhqk,nhkd->nhqd', attn, v)
    o = o.transpose(2, 0, 1, 3).reshape(S, N, E)
    return o @ Wo.T + bo


def kernel(jaf_64, jaf_32, jaf_16, jaf_8, joints_3d, level_params, view_attn, level_attn, refine_params):
    jaf = [jaf_64, jaf_32, jaf_16, jaf_8]
    level_params = [tuple(np.asarray(t, np.float32) for t in tp) for tp in level_params]
    view_attn = tuple(np.asarray(t, np.float32) for t in view_attn)
    level_attn = tuple(np.asarray(t, np.float32) for t in level_attn)
    W1, b1, W2, b2 = (np.asarray(t, np.float32) for t in refine_params)
    joints = np.asarray(joints_3d, np.float32)

    if "nc" not in _STATE:
        _STATE["nc"] = _build()
    nc = _STATE["nc"]

    in_maps = _prep_inputs(jaf, level_params)
    _STATE["in_maps"] = in_maps
    results = run_bass_kernel_spmd(nc, in_maps, list(range(NCORES))).results
    _STATE["results"] = results

    out = np.empty((B, J, 3), np.float32)
    for b in range(NCORES):
        zred = results[b]["zred"]  # [4*84, 512] summed partials, batch b
        level_out = []
        for l in range(4):
            _, bias, g, beta = level_params[l]
            z = zred[l * SLAB:(l + 1) * SLAB] + bias
            h = _layernorm(np.maximum(z, 0.0), g, beta)
            o = _mha(h.reshape(V, J, E), *view_attn)
            level_out.append(o.mean(0))
        L = np.stack(level_out, 0)                      # [4, J, E]
        fused = _mha(L, *level_attn).mean(0)            # [J, E]
        delta = np.maximum(fused @ W1.T + b1, 0.0) @ W2.T + b2
        out[b] = joints[b] + delta
    return out


def hw_exec_time_ns(iters=3):
    """Best-effort HW time. NTFF tracing is unavailable under this axon client,
    so report min warm wall time of the SPMD execution (upper bound: includes
    PJRT input upload of ~450 MB and per-call jit re-trace)."""
    import time
    nc = _STATE["nc"]
    in_maps = _STATE["in_maps"]
    try:
        res = run_bass_kernel_spmd(nc, in_maps, list(range(NCORES)), trace=True)
        if res.exec_time_ns:
            path = res.instructions_and_trace[1] if res.instructions_and_trace else None
            return res.exec_time_ns, path
    except Exception:
        pass
    best = None
    for _ in range(iters):
        t0 = time.perf_counter()
        run_bass_kernel_spmd(nc, in_maps, list(range(NCORES)))
        dt = time.perf_counter() - t0
        best = dt if best is None or dt < best else best
    return int(best * 1e9), "min warm wall of SPMD run (upper bound, incl 450MB upload)"


# revision 11
# speedup vs baseline: 1.0234x; 1.0065x over previous
"""Trainium2 Bass kernel for nn_LAT_16260746182953.

Strategy (8 NeuronCores):
- The dominant cost is the 4 per-level Linears: [672, K_l] @ [K_l, 512] with
  K_l = c*49 up to 100352 (129.5 GFLOP, ~0.9 GB of operands).
- Shard the CONTRACTION dim K 8-ways: each core streams 1/8 of x (transposed,
  bf16) and 1/8 of W (transposed, bf16) and accumulates partial [672, 512]
  sums in PSUM (fp32), then a ReduceScatter(add) sums partials across cores
  and hands core i exactly the rows of batch b=i (672 = 8 batches x 84 rows).
  Per-core traffic: ~56 MB vs ~450 MB for batch-data-parallel.
- All DMAs are issued on gpsimd (SWDGE): HWDGE (nc.sync) DMAs coexisting with
  collectives in one NEFF crash the device (measured).
- Host-side: input layout prep (transpose to [K, 672] p-major chunk blocks,
  bf16 cast) and the tiny epilogue (LayerNorm + two 4-token attentions +
  refine MLP, <0.5% of FLOPs) in exact fp32 numpy.
"""
import numpy as np
import ml_dtypes

import concourse.bacc as bacc
import concourse.mybir as mybir
import concourse.tile as tile
from concourse.bass_utils import run_bass_kernel_spmd
from concourse import bass2jax

NCORES = 8
B, V, J = 8, 4, 21
E = 512
NH = 8
M = B * V * J          # 672 rows of the big matmul
SLAB = M // NCORES     # 84 rows per core after reduce-scatter
CH = 16                # k-tiles (of 128 rows) per DMA chunk
CHANNELS = [256, 512, 1024, 2048]
KFULL = [c * 49 for c in CHANNELS]
KS = [k // NCORES for k in KFULL]           # per-core contraction shard
NT = [(k + 127) // 128 for k in KS]         # zero-padded k-tiles
LEVELS = [3, 2, 1, 0]                       # biggest level first
MT = [(0, 128), (128, 128), (256, 128), (384, 128), (512, 128), (640, 32)]
BF16 = ml_dtypes.bfloat16


def _chunks_of(nt):
    out, o = [], 0
    while o < nt:
        c = min(CH, nt - o)
        out.append((o, c))
        o += c
    return out


_STATE = {}


def _build():
    nc = bacc.Bacc("TRN2", target_bir_lowering=False, debug=False, num_devices=NCORES)
    x_in = {l: nc.dram_tensor(f"x{l}", [NT[l] * 128 * M], mybir.dt.bfloat16, kind="ExternalInput") for l in range(4)}
    w_in = {l: nc.dram_tensor(f"w{l}", [NT[l] * 128 * E], mybir.dt.bfloat16, kind="ExternalInput") for l in range(4)}
    zred = nc.dram_tensor("zred", [4 * SLAB, E], mybir.dt.float32, kind="ExternalOutput")

    with tile.TileContext(nc) as tc:
        with (
            tc.tile_pool(name="xp", bufs=3) as xp,
            tc.tile_pool(name="wp", bufs=3) as wp,
            tc.tile_pool(name="zp", bufs=2) as zp,
            tc.tile_pool(name="acc", bufs=1, space="PSUM") as accp,
            tc.tile_pool(name="dram", bufs=1, space="DRAM") as dram,
        ):
            for l in LEVELS:
                cks = _chunks_of(NT[l])
                psums = [
                    accp.tile([mw, E], mybir.dt.float32, tag=f"acc{mi}", name=f"acc{l}_{mi}")
                    for mi, (mo, mw) in enumerate(MT)
                ]
                for ci, (t0, ct) in enumerate(cks):
                    xt = xp.tile([128, ct * M], mybir.dt.bfloat16, tag="x", name=f"x_{l}_{ci}")
                    nc.gpsimd.dma_start(
                        out=xt[:],
                        in_=x_in[l].ap()[t0 * 128 * M:(t0 + ct) * 128 * M].rearrange("(p f) -> p f", p=128),
                    )
                    wt = wp.tile([128, ct * E], mybir.dt.bfloat16, tag="w", name=f"w_{l}_{ci}")
                    nc.gpsimd.dma_start(
                        out=wt[:],
                        in_=w_in[l].ap()[t0 * 128 * E:(t0 + ct) * 128 * E].rearrange("(p f) -> p f", p=128),
                    )
                    first, last = ci == 0, ci == len(cks) - 1
                    for mi, (mo, mw) in enumerate(MT):
                        for kt in range(ct):
                            nc.tensor.matmul(
                                psums[mi][:],
                                xt[:, kt * M + mo: kt * M + mo + mw],
                                wt[:, kt * E:(kt + 1) * E],
                                start=(first and kt == 0),
                                stop=(last and kt == ct - 1),
                            )
                cc_in = dram.tile([M, E], mybir.dt.float32, tag="cc_in", name=f"cc_in{l}", bufs=2)
                for mi, (mo, mw) in enumerate(MT):
                    zt = zp.tile([mw, E], mybir.dt.float32, tag="z", name=f"z_{l}_{mi}")
                    nc.vector.tensor_copy(zt[:], psums[mi][:])
                    nc.gpsimd.dma_start(out=cc_in[mo:mo + mw, :], in_=zt[:])
                cc_out = dram.tile([SLAB, E], mybir.dt.float32, tag="cc_out", name=f"cc_out{l}", bufs=2)
                nc.gpsimd.collective_compute(
                    "ReduceScatter",
                    mybir.AluOpType.add,
                    replica_groups=[list(range(NCORES))],
                    ins=[cc_in.opt()],
                    outs=[cc_out.opt()],
                )
                nc.gpsimd.dma_start(out=zred.ap()[l * SLAB:(l + 1) * SLAB, :], in_=cc_out[:])

    nc.compile()
    return nc


def _get_runner(nc):
    """Cached jitted shard_map runner (mirrors bass2jax.run_bass_via_pjrt) so
    repeated executions reuse the XLA executable and inputs can stay on device."""
    if "runner" in _STATE:
        return _STATE["runner"]
    import jax
    from jax.experimental.shard_map import shard_map
    from jax.sharding import Mesh, PartitionSpec

    bass2jax.install_neuronx_cc_hook()
    in_names, out_names, out_avals, zero_outs = [], [], [], []
    for alloc in nc.m.functions[0].allocations:
        if not isinstance(alloc, mybir.MemoryLocationSet):
            continue
        name = alloc.memorylocations[0].name
        if alloc.kind == "ExternalInput":
            in_names.append(name)
        elif alloc.kind == "ExternalOutput":
            shape = tuple(alloc.tensor_shape)
            dtype = mybir.dt.np(alloc.dtype)
            out_names.append(name)
            out_avals.append(jax.core.ShapedArray(shape, dtype))
            zero_outs.append(np.zeros(shape, dtype))
    n_params, n_outs = len(in_names), len(out_avals)
    all_names = in_names + out_names

    def _body(*args):
        outs = bass2jax._bass_exec_p.bind(
            *args,
            out_avals=tuple(out_avals),
            in_names=tuple(all_names),
            out_names=tuple(out_names),
            lowering_input_output_aliases=(),
            sim_require_finite=True,
            sim_require_nnan=True,
            nc=nc,
        )
        return tuple(outs)

    devices = jax.devices()[:NCORES]
    mesh = Mesh(np.asarray(devices), ("core",))
    spec = PartitionSpec("core")
    sharded = jax.jit(
        shard_map(_body, mesh=mesh, in_specs=(spec,) * (n_params + n_outs),
                  out_specs=(spec,) * n_outs, check_rep=False),
        donate_argnums=tuple(range(n_params, n_params + n_outs)),
        keep_unused=True,
    )
    _STATE["runner"] = (sharded, in_names, out_names, out_avals, zero_outs, mesh, spec)
    return _STATE["runner"]


def _run_device(nc, in_maps):
    import jax
    sharded, in_names, out_names, out_avals, zero_outs, mesh, spec = _get_runner(nc)
    concat_in = [np.concatenate([in_maps[c][n] for c in range(NCORES)], axis=0) for n in in_names]
    concat_zeros = [np.zeros((NCORES * z.shape[0], *z.shape[1:]), z.dtype) for z in zero_outs]
    out_arrs = sharded(*concat_in, *concat_zeros)
    return [
        {n: np.asarray(out_arrs[i]).reshape(NCORES, *out_avals[i].shape)[c]
         for i, n in enumerate(out_names)}
        for c in range(NCORES)
    ]


def _prep(a_pad, F):
    """[nt*128, F] f32/bf16 -> flat chunk-major, p-major bf16 layout."""
    nt = a_pad.shape[0] // 128
    out = []
    for t0, ct in _chunks_of(nt):
        blk = a_pad[t0 * 128:(t0 + ct) * 128].reshape(ct, 128, F).swapaxes(0, 1)
        out.append(np.ascontiguousarray(blk).reshape(-1))
    return np.concatenate(out)


def _prep_core(a, nt, F):
    """[KS, F] bf16 contiguous -> zero-padded flat chunk-major p-major layout."""
    ks = a.shape[0]
    pad = nt * 128 - ks
    if pad:
        a = np.concatenate([a, np.zeros((pad, F), a.dtype)], 0)
    parts = []
    for t0, ct in _chunks_of(nt):
        blk = a[t0 * 128:(t0 + ct) * 128].reshape(ct, 128, F).swapaxes(0, 1)
        parts.append(np.ascontiguousarray(blk).reshape(-1))
    return np.concatenate(parts)


def _prep_inputs(jaf, level_params):
    in_maps = [{} for _ in range(NCORES)]
    for l in range(4):
        K = KFULL[l]
        # bulk cast to bf16 first (vectorized), then one bf16 transpose per level
        x_bf = np.asarray(jaf[l], np.float32).reshape(M, K).astype(BF16)
        xT = np.ascontiguousarray(x_bf.T)                      # [K, M] bf16
        w_bf = np.asarray(level_params[l][0], np.float32).astype(BF16)
        wT = np.ascontiguousarray(w_bf.T)                      # [K, E] bf16
        for i in range(NCORES):
            sl = slice(i * KS[l], (i + 1) * KS[l])
            in_maps[i][f"x{l}"] = _prep_core(xT[sl], NT[l], M)
            in_maps[i][f"w{l}"] = _prep_core(wT[sl], NT[l], E)
    return in_maps


# ---------------- host epilogue (exact fp32, mirrors the reference) ----------------

def _layernorm(x, g, b):
    mu = x.mean(-1, keepdims=True)
    var = ((x - mu) ** 2).mean(-1, keepdims=True)
    return (x - mu) / np.sqrt(var + 1e-5) * g + b


def _mha(x, Wi, bi, Wo, bo):
    S, N, _ = x.shape
    hd = E // NH
    qkv = x @ Wi.T + bi
    q, k, v = np.split(qkv, 3, axis=-1)

    def heads(t):
        return t.reshape(S, N, NH, hd).transpose(1, 2, 0, 3)

    q, k, v = heads(q), heads(k), heads(v)
    scores = np.einsum('nhqd,nhkd->nhqk', q, k) / np.sqrt(np.float32(hd))
    scores = scores - scores.max(-1, keepdims=True)
    ex = np.exp(scores)
    attn = ex / ex.sum(-1, keepdims=True)
    o = np.einsum('nhqk,nhkd->nhqd', attn, v)
    o = o.transpose(2, 0, 1, 3).reshape(S, N, E)
    return o @ Wo.T + bo


def kernel(jaf_64, jaf_32, jaf_16, jaf_8, joints_3d, level_params, view_attn, level_attn, refine_params):
    jaf = [jaf_64, jaf_32, jaf_16, jaf_8]
    level_params = [tuple(np.asarray(t, np.float32) for t in tp) for tp in level_params]
    view_attn = tuple(np.asarray(t, np.float32) for t in view_attn)
    level_attn = tuple(np.asarray(t, np.float32) for t in level_attn)
    W1, b1, W2, b2 = (np.asarray(t, np.float32) for t in refine_params)
    joints = np.asarray(joints_3d, np.float32)

    if "nc" not in _STATE:
        _STATE["nc"] = _build()
    nc = _STATE["nc"]

    in_maps = _prep_inputs(jaf, level_params)
    _STATE["in_maps"] = in_maps
    results = run_bass_kernel_spmd(nc, in_maps, list(range(NCORES))).results
    _STATE["results"] = results

    out = np.empty((B, J, 3), np.float32)
    for b in range(NCORES):
        zred = results[b]["zred"]  # [4*84, 512] summed partials, batch b
        level_out = []
        for l in range(4):
            _, bias, g, beta = level_params[l]
            z = zred[l * SLAB:(l + 1) * SLAB] + bias
            h = _layernorm(np.maximum(z, 0.0), g, beta)
            o = _mha(h.reshape(V, J, E), *view_attn)
            level_out.append(o.mean(0))
        L = np.stack(level_out, 0)                      # [4, J, E]
        fused = _mha(L, *level_attn).mean(0)            # [J, E]
        delta = np.maximum(fused @ W1.T + b1, 0.0) @ W2.T + b2
        out[b] = joints[b] + delta
    return out


def hw_exec_time_ns(iters=3):
    """Best-effort HW time. NTFF tracing is unavailable under this axon client,
    so report min warm wall time of the SPMD execution (upper bound: includes
    PJRT input upload of ~450 MB and per-call jit re-trace)."""
    import time
    nc = _STATE["nc"]
    in_maps = _STATE["in_maps"]
    try:
        res = run_bass_kernel_spmd(nc, in_maps, list(range(NCORES)), trace=True)
        if res.exec_time_ns:
            path = res.instructions_and_trace[1] if res.instructions_and_trace else None
            return res.exec_time_ns, path
    except Exception:
        pass
    best = None
    for _ in range(iters):
        t0 = time.perf_counter()
        run_bass_kernel_spmd(nc, in_maps, list(range(NCORES)))
        dt = time.perf_counter() - t0
        best = dt if best is None or dt < best else best
    return int(best * 1e9), "min warm wall of SPMD run (upper bound, incl 450MB upload)"
